# revision 20
# baseline (speedup 1.0000x reference)
"""Block-diagonal (local) attention kernel for Trainium2, 8-core SPMD.

Problem: q, k, v = [8, 16, 4096, 128] fp32; block_size=128 local attention.
Per 128-token block: score = qb @ kb.T (no 1/sqrt(D) scaling), softmax over
keys, out = probs @ vb.  Blocks are independent -> shard batch across the 8
NeuronCores, no cross-device communication.

Design log (fp32 baseline ~526us -> v2 228 -> v3/v4 ~202 -> this):
  - Host pre-transposes q,k into [d, w] block layout: no PE transposes.
  - 16-bit wire: q/k/v fp16, probs bf16 (needs fp32 exponent range:
    exp(s-25) reaches ~1e17), out fp16.  Halves HBM bytes, 4x PE rate.
  - Per half-head chunk (16 blocks): score matmuls into PSUM, exp batched
    8 blocks per ACTIVATE ([128,1024], amortizes ACT's 172-cycle fixed
    cost), PV matmuls 3 blocks per PSUM bank with a host-baked ones
    column in v producing the softmax denominator for free, then
    normalize STRAIGHT out of PSUM: reciprocal of the denominator column
    + broadcast tensor_mul -> fp16 output tile (no staging copies).
    One PV group per chunk normalizes on ACT (per-block scale-copy,
    scale = per-partition reciprocal vector) to balance ACT/DVE.
  - Input split in two DMAs: qk tile is released by the score matmuls
    (early), the small v tile is held until PV.  A combined tile made
    input prefetch wait on the previous chunk's LAST PV matmul, eroding
    the DMA lead until every chunk paid full transfer latency.
  - Output DMA triggers issue from the (otherwise idle) gpsimd queue:
    on the sync queue their wait-for-output blocked the input prefetch
    stream.  Software-pipelined program order (chunk cc scores emitted
    before chunk cc-1 PV) hides exp latency from the in-order PE queue.

Bottleneck: HBM DMA (~64 MiB/core, ~420 GB/s observed sustained).
"""

import numpy as np

import concourse.bass as bass
import concourse.tile as tile
from concourse import bacc, bass_utils, mybir

B = 8
H = 16
L = 4096
D = 128
W = 128            # attention block size
NB = L // W        # blocks per head (32)
N_CORES = 8
EXP_SHIFT = -25.0

CNB = 16           # blocks per chunk (= half a head)
N_CHUNKS = (H * NB) // CNB
QK_COLS = 2 * CNB * W      # 4096: qT | kT
V_COLS = CNB * (D + 1)     # 2064: v with ones column baked in per block
EG = 8             # blocks per exp group (two PSUM banks of scores)
PG = 3             # blocks per PV group (3*129 <= 512 psum cols)
ACT_GROUP = None   # ACT normalize offload disabled: cross-engine waits
                   # head-of-line block the in-order scalar queue


def build_bass(num_devices: int = N_CORES) -> bass.Bass:
    f16 = mybir.dt.float16
    bf16 = mybir.dt.bfloat16
    f32 = mybir.dt.float32
    nc = bacc.Bacc(
        "TRN2", target_bir_lowering=False, debug=False, num_devices=num_devices
    )
    xqk = nc.dram_tensor(
        "xqk", (N_CHUNKS * 128, QK_COLS), f16, kind="ExternalInput"
    ).ap()
    xv = nc.dram_tensor(
        "xv", (N_CHUNKS * 128, V_COLS), f16, kind="ExternalInput"
    ).ap()
    o = nc.dram_tensor(
        "out", (N_CHUNKS * 128, CNB * D), f16, kind="ExternalOutput"
    ).ap()

    with tile.TileContext(nc) as tc:
        with (
            tc.tile_pool(name="pqk", bufs=9) as pqk,
            tc.tile_pool(name="pv", bufs=14) as pv,
            tc.tile_pool(name="po", bufs=6) as po,
            tc.tile_pool(name="probs", bufs=8) as probs,
            tc.tile_pool(name="small", bufs=8) as small,
            tc.tile_pool(name="const", bufs=1) as const,
            tc.tile_pool(name="ps_s", bufs=2, space="PSUM") as ps_s,
            tc.tile_pool(name="ps_o", bufs=2, space="PSUM") as ps_o,
        ):
            exp_bias = const.tile([128, 1], f32)
            nc.gpsimd.memset(exp_bias, EXP_SHIFT)

            def score_exp(cc):
                """Input DMAs + score matmuls + batched exp for chunk cc.
                Returns (vt, pTs) needed by the PV phase."""
                qk = pqk.tile([128, QK_COLS], f16, tag="qk")
                nc.sync.dma_start(out=qk, in_=xqk[cc * 128 : (cc + 1) * 128])
                vt = pv.tile([128, V_COLS], f16, tag="vt")
                nc.sync.dma_start(out=vt, in_=xv[cc * 128 : (cc + 1) * 128])
                pTs = []
                for g in range(CNB // EG):
                    sT = ps_s.tile([128, EG * W], f32, tag="sT")
                    for i in range(EG):
                        n = g * EG + i
                        # sT[u, w] = k[u,:] . q[w,:]
                        nc.tensor.matmul(
                            sT[:, i * W : (i + 1) * W],
                            qk[:, (CNB + n) * W : (CNB + n + 1) * W],
                            qk[:, n * W : (n + 1) * W],
                        )
                    pT = probs.tile([128, EG * W], bf16, tag="pT")
                    nc.scalar.activation(
                        pT, sT, mybir.ActivationFunctionType.Exp,
                        bias=exp_bias, scale=1.0,
                    )
                    pTs.append(pT)
                return vt, pTs

            def pv_norm_out(cc, vt, pTs):
                """PV matmuls, 6 blocks per two-bank PSUM tile (3 blocks
                per 2KB bank, 387B used + pad so no matmul output crosses a
                bank), then ONE strided reciprocal of the 6 denominator
                columns and ONE broadcast tensor_mul PSUM->SBUF fp16 per
                tile.  Halves the DVE instruction count per chunk."""
                oh = po.tile([128, CNB, D], f16, tag="oh")
                for n0 in range(0, CNB, 2 * PG):
                    nn = min(2 * PG, CNB - n0)
                    nj = nn // 2            # blocks per bank half
                    o_ps = ps_o.tile([128, 2, 512], f32, tag="o_ps")
                    for jj in range(nn):
                        n = n0 + jj
                        h, j = divmod(jj, nj)
                        pT = pTs[n // EG][:, (n % EG) * W : (n % EG + 1) * W]
                        # out[w, 0:D] = probs @ vb ; out[w, D] = exp row sum
                        nc.tensor.matmul(
                            o_ps[:, h, j * (D + 1) : (j + 1) * (D + 1)],
                            pT,
                            vt[:, n * (D + 1) : (n + 1) * (D + 1)],
                        )
                    # 4D view [p, half, block, col] of the used region
                    view = o_ps[:, :, 0 : nj * (D + 1)].rearrange(
                        "p h (j c) -> p h j c", c=D + 1
                    )
                    r = small.tile([128, 2, nj, 1], f32, tag=f"r{nj}")
                    nc.vector.reciprocal(r, view[:, :, :, D : D + 1])
                    nc.vector.tensor_mul(
                        oh[:, n0 : n0 + nn, :],
                        view[:, :, :, 0:D],
                        r.broadcast_to([128, 2, nj, D]),
                    )
                nc.gpsimd.dma_start(
                    out=o[cc * 128 : (cc + 1) * 128], in_=oh
                )

            # software pipelining: emit chunk cc's scores before chunk
            # cc-1's PV phase
            prev = None
            for cc in range(N_CHUNKS):
                cur = (cc, *score_exp(cc))
                if prev is not None:
                    pv_norm_out(*prev)
                prev = cur
            pv_norm_out(*prev)

    nc.compile()
    return nc


_nc_cache = None


def _get_nc() -> bass.Bass:
    global _nc_cache
    if _nc_cache is None:
        _nc_cache = build_bass()
    return _nc_cache


def _pack_inputs(q, k, v):
    """Pack one batch's q,k,v [H,L,D] fp32 into device layouts:
    xqk [N_CHUNKS*128, QK_COLS] (qT | kT per chunk) and
    xv [N_CHUNKS*128, V_COLS] (v with ones column per block), both fp16."""
    xqk = np.empty((N_CHUNKS, 128, QK_COLS), dtype=np.float16)
    half = CNB * W
    # (cc, p, w, d) -> (cc, d, p, w)
    xqk[:, :, :half] = (
        q.reshape(N_CHUNKS, CNB, W, D).transpose(0, 3, 1, 2).reshape(N_CHUNKS, D, half)
    )
    xqk[:, :, half:] = (
        k.reshape(N_CHUNKS, CNB, W, D).transpose(0, 3, 1, 2).reshape(N_CHUNKS, D, half)
    )
    xv = np.empty((N_CHUNKS, 128, CNB, D + 1), dtype=np.float16)
    # (cc, p, u, d) -> (cc, u, p, d)
    xv[:, :, :, :D] = v.reshape(N_CHUNKS, CNB, W, D).transpose(0, 2, 1, 3)
    xv[:, :, :, D] = 1.0
    return (
        xqk.reshape(N_CHUNKS * 128, QK_COLS),
        xv.reshape(N_CHUNKS * 128, V_COLS),
    )


def _prepare_in_maps(q, k, v):
    q = np.asarray(q, dtype=np.float32)
    k = np.asarray(k, dtype=np.float32)
    v = np.asarray(v, dtype=np.float32)
    assert q.shape == (B, H, L, D), q.shape
    maps = []
    for b in range(B):
        xqk, xv = _pack_inputs(q[b], k[b], v[b])
        maps.append({"xqk": xqk, "xv": xv})
    return maps


def _unpack_out(o: np.ndarray) -> np.ndarray:
    """[N_CHUNKS*128, CNB*D] fp16 -> [H, L, D] fp32."""
    return (
        o.reshape(N_CHUNKS, W, CNB, D).transpose(0, 2, 1, 3).reshape(H, L, D)
    ).astype(np.float32)


def kernel(**inputs: np.ndarray) -> np.ndarray:
    nc = _get_nc()
    in_maps = _prepare_in_maps(inputs["q"], inputs["k"], inputs["v"])
    res = bass_utils.run_bass_kernel_spmd(nc, in_maps, core_ids=list(range(N_CORES)))
    return np.stack(
        [_unpack_out(np.asarray(res.results[b]["out"])) for b in range(B)], axis=0
    )


# revision 21
# speedup vs baseline: 1.0052x; 1.0052x over previous
"""Block-diagonal (local) attention kernel for Trainium2, 8-core SPMD.

Problem: q, k, v = [8, 16, 4096, 128] fp32; block_size=128 local attention.
Per 128-token block: score = qb @ kb.T (no 1/sqrt(D) scaling), softmax over
keys, out = probs @ vb.  Blocks are independent -> shard batch across the 8
NeuronCores, no cross-device communication.

Design log (fp32 baseline ~526us -> v2 228 -> v3/v4 ~202 -> this):
  - Host pre-transposes q,k into [d, w] block layout: no PE transposes.
  - 16-bit wire: q/k/v fp16, probs bf16 (needs fp32 exponent range:
    exp(s-25) reaches ~1e17), out fp16.  Halves HBM bytes, 4x PE rate.
  - Per half-head chunk (16 blocks): score matmuls into PSUM, exp batched
    8 blocks per ACTIVATE ([128,1024], amortizes ACT's 172-cycle fixed
    cost), PV matmuls 3 blocks per PSUM bank with a host-baked ones
    column in v producing the softmax denominator for free, then
    normalize STRAIGHT out of PSUM: reciprocal of the denominator column
    + broadcast tensor_mul -> fp16 output tile (no staging copies).
    One PV group per chunk normalizes on ACT (per-block scale-copy,
    scale = per-partition reciprocal vector) to balance ACT/DVE.
  - Input split in two DMAs: qk tile is released by the score matmuls
    (early), the small v tile is held until PV.  A combined tile made
    input prefetch wait on the previous chunk's LAST PV matmul, eroding
    the DMA lead until every chunk paid full transfer latency.
  - Output DMA triggers issue from the (otherwise idle) gpsimd queue:
    on the sync queue their wait-for-output blocked the input prefetch
    stream.  Software-pipelined program order (chunk cc scores emitted
    before chunk cc-1 PV) hides exp latency from the in-order PE queue.

Bottleneck: HBM DMA (~64 MiB/core, ~420 GB/s observed sustained).
"""

import numpy as np

import concourse.bass as bass
import concourse.tile as tile
from concourse import bacc, bass_utils, mybir

B = 8
H = 16
L = 4096
D = 128
W = 128            # attention block size
NB = L // W        # blocks per head (32)
N_CORES = 8
EXP_SHIFT = -25.0

CNB = 16           # blocks per chunk (= half a head)
N_CHUNKS = (H * NB) // CNB
QK_COLS = 2 * CNB * W      # 4096: qT | kT
V_COLS = CNB * (D + 1)     # 2064: v with ones column baked in per block
EG = 4             # blocks per exp group (one PSUM bank of scores)
PG = 3             # blocks per PV group (3*129 <= 512 psum cols)
ACT_GROUP = None   # ACT normalize offload disabled: cross-engine waits
                   # head-of-line block the in-order scalar queue


def build_bass(num_devices: int = N_CORES) -> bass.Bass:
    f16 = mybir.dt.float16
    bf16 = mybir.dt.bfloat16
    f32 = mybir.dt.float32
    nc = bacc.Bacc(
        "TRN2", target_bir_lowering=False, debug=False, num_devices=num_devices
    )
    xqk = nc.dram_tensor(
        "xqk", (N_CHUNKS * 128, QK_COLS), f16, kind="ExternalInput"
    ).ap()
    xv = nc.dram_tensor(
        "xv", (N_CHUNKS * 128, V_COLS), f16, kind="ExternalInput"
    ).ap()
    o = nc.dram_tensor(
        "out", (N_CHUNKS * 128, CNB * D), f16, kind="ExternalOutput"
    ).ap()

    with tile.TileContext(nc) as tc:
        with (
            tc.tile_pool(name="pqk", bufs=9) as pqk,
            tc.tile_pool(name="pv", bufs=14) as pv,
            tc.tile_pool(name="po", bufs=6) as po,
            tc.tile_pool(name="probs", bufs=8) as probs,
            tc.tile_pool(name="small", bufs=8) as small,
            tc.tile_pool(name="const", bufs=1) as const,
            tc.tile_pool(name="ps_s", bufs=2, space="PSUM") as ps_s,
            tc.tile_pool(name="ps_o", bufs=3, space="PSUM") as ps_o,
        ):
            exp_bias = const.tile([128, 1], f32)
            nc.gpsimd.memset(exp_bias, EXP_SHIFT)

            def score_exp(cc):
                """Input DMAs + score matmuls + batched exp for chunk cc.
                Returns (vt, pTs) needed by the PV phase."""
                qk = pqk.tile([128, QK_COLS], f16, tag="qk")
                nc.sync.dma_start(out=qk, in_=xqk[cc * 128 : (cc + 1) * 128])
                vt = pv.tile([128, V_COLS], f16, tag="vt")
                nc.sync.dma_start(out=vt, in_=xv[cc * 128 : (cc + 1) * 128])
                pTs = []
                for g in range(CNB // EG):
                    sT = ps_s.tile([128, EG * W], f32, tag="sT")
                    for i in range(EG):
                        n = g * EG + i
                        # sT[u, w] = k[u,:] . q[w,:]
                        nc.tensor.matmul(
                            sT[:, i * W : (i + 1) * W],
                            qk[:, (CNB + n) * W : (CNB + n + 1) * W],
                            qk[:, n * W : (n + 1) * W],
                        )
                    pT = probs.tile([128, EG * W], bf16, tag="pT")
                    nc.scalar.activation(
                        pT, sT, mybir.ActivationFunctionType.Exp,
                        bias=exp_bias, scale=1.0,
                    )
                    pTs.append(pT)
                return vt, pTs

            def pv_norm_out(cc, vt, pTs):
                """PV matmuls, 6 blocks per two-bank PSUM tile (3 blocks
                per 2KB bank, 387B used + pad so no matmul output crosses a
                bank), then ONE strided reciprocal of the 6 denominator
                columns and ONE broadcast tensor_mul PSUM->SBUF fp16 per
                tile.  Halves the DVE instruction count per chunk."""
                oh = po.tile([128, CNB, D], f16, tag="oh")
                for n0 in range(0, CNB, 2 * PG):
                    nn = min(2 * PG, CNB - n0)
                    nj = nn // 2            # blocks per bank half
                    o_ps = ps_o.tile([128, 2, 512], f32, tag="o_ps")
                    for jj in range(nn):
                        n = n0 + jj
                        h, j = divmod(jj, nj)
                        pT = pTs[n // EG][:, (n % EG) * W : (n % EG + 1) * W]
                        # out[w, 0:D] = probs @ vb ; out[w, D] = exp row sum
                        nc.tensor.matmul(
                            o_ps[:, h, j * (D + 1) : (j + 1) * (D + 1)],
                            pT,
                            vt[:, n * (D + 1) : (n + 1) * (D + 1)],
                        )
                    # 4D view [p, half, block, col] of the used region
                    view = o_ps[:, :, 0 : nj * (D + 1)].rearrange(
                        "p h (j c) -> p h j c", c=D + 1
                    )
                    r = small.tile([128, 2, nj, 1], f32, tag=f"r{nj}")
                    nc.vector.reciprocal(r, view[:, :, :, D : D + 1])
                    nc.vector.tensor_mul(
                        oh[:, n0 : n0 + nn, :],
                        view[:, :, :, 0:D],
                        r.broadcast_to([128, 2, nj, D]),
                    )
                nc.gpsimd.dma_start(
                    out=o[cc * 128 : (cc + 1) * 128], in_=oh
                )

            # software pipelining: emit chunk cc's scores before chunk
            # cc-1's PV phase
            prev = None
            for cc in range(N_CHUNKS):
                cur = (cc, *score_exp(cc))
                if prev is not None:
                    pv_norm_out(*prev)
                prev = cur
            pv_norm_out(*prev)

    nc.compile()
    return nc


_nc_cache = None


def _get_nc() -> bass.Bass:
    global _nc_cache
    if _nc_cache is None:
        _nc_cache = build_bass()
    return _nc_cache


def _pack_inputs(q, k, v):
    """Pack one batch's q,k,v [H,L,D] fp32 into device layouts:
    xqk [N_CHUNKS*128, QK_COLS] (qT | kT per chunk) and
    xv [N_CHUNKS*128, V_COLS] (v with ones column per block), both fp16."""
    xqk = np.empty((N_CHUNKS, 128, QK_COLS), dtype=np.float16)
    half = CNB * W
    # (cc, p, w, d) -> (cc, d, p, w)
    xqk[:, :, :half] = (
        q.reshape(N_CHUNKS, CNB, W, D).transpose(0, 3, 1, 2).reshape(N_CHUNKS, D, half)
    )
    xqk[:, :, half:] = (
        k.reshape(N_CHUNKS, CNB, W, D).transpose(0, 3, 1, 2).reshape(N_CHUNKS, D, half)
    )
    xv = np.empty((N_CHUNKS, 128, CNB, D + 1), dtype=np.float16)
    # (cc, p, u, d) -> (cc, u, p, d)
    xv[:, :, :, :D] = v.reshape(N_CHUNKS, CNB, W, D).transpose(0, 2, 1, 3)
    xv[:, :, :, D] = 1.0
    return (
        xqk.reshape(N_CHUNKS * 128, QK_COLS),
        xv.reshape(N_CHUNKS * 128, V_COLS),
    )


def _prepare_in_maps(q, k, v):
    q = np.asarray(q, dtype=np.float32)
    k = np.asarray(k, dtype=np.float32)
    v = np.asarray(v, dtype=np.float32)
    assert q.shape == (B, H, L, D), q.shape
    maps = []
    for b in range(B):
        xqk, xv = _pack_inputs(q[b], k[b], v[b])
        maps.append({"xqk": xqk, "xv": xv})
    return maps


def _unpack_out(o: np.ndarray) -> np.ndarray:
    """[N_CHUNKS*128, CNB*D] fp16 -> [H, L, D] fp32."""
    return (
        o.reshape(N_CHUNKS, W, CNB, D).transpose(0, 2, 1, 3).reshape(H, L, D)
    ).astype(np.float32)


def kernel(**inputs: np.ndarray) -> np.ndarray:
    nc = _get_nc()
    in_maps = _prepare_in_maps(inputs["q"], inputs["k"], inputs["v"])
    res = bass_utils.run_bass_kernel_spmd(nc, in_maps, core_ids=list(range(N_CORES)))
    return np.stack(
        [_unpack_out(np.asarray(res.results[b]["out"])) for b in range(B)], axis=0
    )
